# revision 25
# baseline (speedup 1.0000x reference)
"""Trainium2 8-core kernel for MemoryEfficientAttention.

Full multi-head attention layer: Q/K/V projections + exact-softmax attention
+ output projection for [B=4, S=2048, D=1024], H=16 heads, dk=64.

Sharding: core c handles batch c//2 and head-half c%2 (8 heads = 512 dims).
Each core produces a partial out-projection [2048, 1024] in bf16; the host
sums the two partials per batch (fp32) and adds the output bias.

Per-core dataflow (V projection fp32r, Q/K/attention bf16, PSUM fp32):
  x arrives pre-transposed from the host as xT [1024 d, 2048 tok]
  (xq/xk bf16, xv fp32).
  QT[jt] = Wq[:,jt]^T @ xqT  [128 j, 2048 q] bf16 (8 dt-accumulated bf16
  KT[jt] likewise            matmuls per half; ScalarE bias-add)
  V[kt]  = xvT[kt]^T @ Wv    [128 tok, 8 heads, 64+1] bf16 natural layout
                             with a ones column per head (softmax denom)
  per unit (q-half, head):   16 units, each a 16-k-tile loop:
    sT  = K_h @ Q_h          [128 k, 1024 q] PSUM (K=64 contraction),
                             double-buffered so scores run ahead of exp
    eT  = exp(sT)            ScalarE [128,1024], PSUM -> bf16 SBUF
    av += V_aug^T @ eT       [65, 1024] PSUM accumulated over 16 k-tiles;
                             row 64 = softmax denominator
    (projection / out-projection "filler" closures are interleaved into
     the k-tile loop so the PE never idles long enough to lose the HAM
     clock boost)
  oc  = av[0:64] * (1/av[64])  DVE reciprocal ([64,16] reshape via DRAM
                             bounce) + row-broadcast DMA + DVE mult
  y   = oc^T @ Wo            [2048, 1024] bf16 partial, accumulated over jt
"""

import numpy as np

import concourse.bass as bass
import concourse.mybir as mybir
import concourse.tile as tile
from concourse import bacc

B, S, D, H, DK = 4, 2048, 1024, 16, 64
NCORES = 8
HPC = H // 2          # heads per core
DH = HPC * DK         # 512 projection dims per core
NJT = 4               # head pairs per core
NDT = D // 128        # 8 d-tiles
NKT = S // 128        # 16 k-tiles
F32 = mybir.dt.float32
F32R = mybir.dt.float32r
BF16 = mybir.dt.bfloat16
U16 = mybir.dt.uint16
EXP = mybir.ActivationFunctionType.Exp
IDENT = mybir.ActivationFunctionType.Identity
MULT = mybir.AluOpType.mult
ADD = mybir.AluOpType.add

# pair-Schraudolph constants (bf16 bit trick; dormant unless USE_SCH)
SCH_A = 128.0 / np.log(2.0)
SCH_D = np.log(2.0) / 4.0
SCH_C = 0.3
SCH_B1 = 16256.0 - SCH_C - SCH_A * SCH_D
SCH_B2 = 16256.0 - SCH_C + SCH_A * SCH_D
EXP_BIAS = 0.743598

USE_SCH = False
SCORE_N1024 = False    # single N=1024 score matmul (else 2x512 into same tile)
ATTNV_N1024 = False    # single N=1024 attnV matmul


def _sch_tile(hh, kt):
    return USE_SCH and hh == 1 and kt % 3 == 1


def _r(ap):
    return ap.bitcast(F32R)


def _bcast_rows(ap_row, nrows):
    """AP that reads one partition row `nrows` times (partition step 0)."""
    return bass.AP(
        tensor=ap_row.tensor,
        offset=ap_row.offset,
        ap=[[0, nrows]] + [list(x) for x in ap_row.ap[1:]],
    )


def _emit(nc, tc, ctx):
    xq = nc.dram_tensor("xq", [D, S], BF16, kind="ExternalInput").ap()
    xk = nc.dram_tensor("xk", [D, S], BF16, kind="ExternalInput").ap()
    xv = nc.dram_tensor("xv", [D, S], BF16, kind="ExternalInput").ap()
    wq = nc.dram_tensor("wq", [D, DH], BF16, kind="ExternalInput").ap()
    wk = nc.dram_tensor("wk", [D, DH], BF16, kind="ExternalInput").ap()
    wv = nc.dram_tensor("wv", [D, DH], BF16, kind="ExternalInput").ap()
    wo = nc.dram_tensor("wo", [DH, D], BF16, kind="ExternalInput").ap()
    bq = nc.dram_tensor("bq", [DH], F32, kind="ExternalInput").ap()
    bk = nc.dram_tensor("bk", [DH], F32, kind="ExternalInput").ap()
    bv = nc.dram_tensor("bv", [DH], F32, kind="ExternalInput").ap()
    y = nc.dram_tensor("y", [S, D], BF16, kind="ExternalOutput").ap()

    consts = ctx.enter_context(tc.tile_pool(name="consts", bufs=1))
    wpool = ctx.enter_context(tc.tile_pool(name="weights", bufs=2))
    xvpool = ctx.enter_context(tc.tile_pool(name="xvchunks", bufs=2))
    xqkpool = ctx.enter_context(tc.tile_pool(name="xqk", bufs=1))
    qkpool = ctx.enter_context(tc.tile_pool(name="qk", bufs=1))
    vpool = ctx.enter_context(tc.tile_pool(name="vps", bufs=1))
    ocpool = ctx.enter_context(tc.tile_pool(name="ocp", bufs=1))
    epool = ctx.enter_context(tc.tile_pool(name="expt", bufs=3))
    smalls = ctx.enter_context(tc.tile_pool(name="smalls", bufs=2))
    ypool = ctx.enter_context(tc.tile_pool(name="ystage", bufs=2))
    dramp = ctx.enter_context(tc.tile_pool(name="drams", bufs=1, space="DRAM"))
    psum = ctx.enter_context(tc.tile_pool(name="psum", bufs=1, space="PSUM"))
    if USE_SCH:
        upool = ctx.enter_context(tc.tile_pool(name="u16", bufs=2))

    # PSUM: tag s = [128,1024] x3 (6 banks; scores rotate + proj/outproj
    # fillers borrow), av = [128,1024] (2 banks)
    def ps_s(name):
        return psum.tile([128, 1024], F32, tag="s", name=name, bufs=3)

    def ps_av(name):
        return psum.tile([128, 1024], F32, tag="av", name=name, bufs=1)

    def ps_aux(name):
        return psum.tile([128, 1024], F32, tag="s", name=name, bufs=3)[:, 0:512]

    ebias = consts.tile([128, 1], F32)
    nc.vector.memset(ebias, EXP_BIAS if USE_SCH else 0.0)

    qt_t = [qkpool.tile([128, S], BF16, tag=f"q{jt}", name=f"qT{jt}")
            for jt in range(NJT)]
    # K pair layout, per head: [128, 8 kt-pairs, 128]; rows 0:64 = even
    # k-tile, rows 64:128 = odd k-tile (enables concurrent row-tiled scores)
    kh_t = [qkpool.tile([128, NKT // 2, 128], BF16, tag=f"k{h}",
                        name=f"kh{h}") for h in range(HPC)]
    v_t = [vpool.tile([128, HPC, DK + 2], BF16, tag=f"v{kt}", name=f"v{kt}")
           for kt in range(NKT)]
    oc_t = [ocpool.tile([128, S], BF16, tag=f"oc{jt}", name=f"oc{jt}")
            for jt in range(NJT)]

    # ---- input staging ----
    # xv: fp32 -> f32r chunks of 256 tokens (gpsimd casting queue), rotated
    # xk/xq: bf16, fully resident (sync queue)
    wv_sb = wpool.tile([128, NDT, DH], BF16, tag="w", name="w_v", bufs=1)
    wv_r = wv.rearrange("(n p) j -> p n j", p=128)
    for dh in range(2):
        nc.scalar.dma_start(out=wv_sb[:, dh * 4:(dh + 1) * 4, :],
                            in_=wv_r[:, dh * 4:(dh + 1) * 4, :])
    wk_sb = wpool.tile([128, NDT, DH], BF16, tag="wqk", name="w_k", bufs=1)
    nc.scalar.dma_start(out=wk_sb, in_=wk.rearrange("(n p) j -> p n j", p=128))
    wq_sb = wpool.tile([128, NDT, DH], BF16, tag="wqk2", name="w_q", bufs=1)
    nc.scalar.dma_start(out=wq_sb, in_=wq.rearrange("(n p) j -> p n j", p=128))

    bq_sb = consts.tile([128, NJT], F32)
    nc.sync.dma_start(out=bq_sb, in_=bq.rearrange("(a p) -> p a", p=128))
    bk_sb = consts.tile([128, NJT], F32)
    nc.sync.dma_start(out=bk_sb, in_=bk.rearrange("(a p) -> p a", p=128))
    bv_row = consts.tile([128, DH], F32)
    nc.sync.dma_start(
        out=bv_row,
        in_=bass.AP(tensor=bv.tensor, offset=bv.offset, ap=[[0, 128], [1, DH]]),
    )

    xk_sb = xqkpool.tile([128, NDT, S], BF16, tag="xk", name="xk_sb")
    xk_r = xk.rearrange("(n p) t -> p n t", p=128)
    xq_sb = xqkpool.tile([128, NDT, S], BF16, tag="xq", name="xq_sb")
    xq_r = xq.rearrange("(n p) t -> p n t", p=128)
    for dp in range(4):
        nc.sync.dma_start(out=xk_sb[:, dp * 2:(dp + 1) * 2, :],
                          in_=xk_r[:, dp * 2:(dp + 1) * 2, :])
    for dp in range(4):
        dq = nc.gpsimd if dp < 2 else nc.sync
        dq.dma_start(out=xq_sb[:, dp * 2:(dp + 1) * 2, :],
                     in_=xq_r[:, dp * 2:(dp + 1) * 2, :])

    # ---- V projection (prologue): 16 chunks of 128 tokens ----
    for ch in range(16):
        xc = xvpool.tile([128, NDT, 128], BF16, tag="xv", name=f"xv{ch}")
        nc.gpsimd.dma_start(
            out=xc,
            in_=xv[:, ch * 128:(ch + 1) * 128].rearrange(
                "(n p) t -> p n t", p=128))
        for ktl in range(1):
            kt = ch
            pv = ps_aux(f"pv{kt}")
            for dt in range(NDT):
                nc.tensor.matmul(
                    pv[:],
                    lhsT=xc[:, dt, :],
                    rhs=wv_sb[:, dt, :],
                    start=(dt == 0),
                    stop=(dt == NDT - 1),
                )
            vt = v_t[kt]
            nc.vector.memset(vt[:, :, DK:DK + 1], 1.0)
            nc.vector.tensor_add(
                out=vt[:, :, 0:DK],
                in0=pv.rearrange("p (h d) -> p h d", h=HPC),
                in1=bv_row.rearrange("p (h d) -> p h d", h=HPC),
            )

    # ---- K / Q projection closures (bf16) ----
    def qk_seg(w_sb, x_sb, b_sb, out_t, jt, seg):
        pq = ps_aux(f"p{out_t[jt].tensor.name}{seg}")
        for dt in range(NDT):
            nc.tensor.matmul(
                pq[:],
                lhsT=w_sb[:, dt, jt * 128:(jt + 1) * 128],
                rhs=x_sb[:, dt, seg * 512:(seg + 1) * 512],
                start=(dt == 0),
                stop=(dt == NDT - 1),
            )
        nc.scalar.activation(
            out=out_t[jt][:, seg * 512:(seg + 1) * 512],
            in_=pq[:],
            func=IDENT,
            bias=b_sb[:, jt:jt + 1],
        )

    def k_closure(jt):
        def emit():
            for seg in range(4):
                pq = ps_aux(f"pk{jt}{seg}")
                for dt in range(NDT):
                    nc.tensor.matmul(
                        pq[:],
                        lhsT=wk_sb[:, dt, jt * 128:(jt + 1) * 128],
                        rhs=xk_sb[:, dt, seg * 512:(seg + 1) * 512],
                        start=(dt == 0),
                        stop=(dt == NDT - 1),
                    )
                stage = smalls.tile([128, 4, 128], BF16, tag="kstg",
                                    name="kstg", bufs=2)
                nc.scalar.activation(
                    out=stage[:], in_=pq.rearrange("p (a b) -> p a b", a=4),
                    func=IDENT, bias=bk_sb[:, jt:jt + 1])
                # scatter: head hh quadrants -> kh_t[2*jt+hh] pair layout
                for hh in range(2):
                    h = 2 * jt + hh
                    r0 = hh * 64
                    for par in range(2):   # kt parity within the seg
                        nc.sync.dma_start(
                            out=kh_t[h][par * 64:par * 64 + 64,
                                        2 * seg:2 * seg + 2, :],
                            in_=stage[r0:r0 + 64, par::2, :],
                        )
        return emit

    def q_closure(jt, qh):
        def emit():
            for seg in (2 * qh, 2 * qh + 1):
                qk_seg(wq_sb, xq_sb, bq_sb, qt_t, jt, seg)
        return emit

    # prologue: K[jt0] full, Q[jt0, qh0]
    k_closure(0)()
    q_closure(0, 0)()

    wo_sb = wpool.tile([128, NJT, D], BF16, tag="wo", name="w_o", bufs=1)
    nc.scalar.dma_start(out=wo_sb, in_=wo.rearrange("(n p) j -> p n j", p=128))

    # ---- out-projection closure (one q-row-tile) ----
    def outproj(qt):
        def emit():
            py = psum.tile([128, 1024], F32, tag="s", name=f"py{qt}", bufs=3)
            for nb in range(2):
                for jt in range(NJT):
                    nc.tensor.matmul(
                        py[:, nb * 512:(nb + 1) * 512],
                        lhsT=oc_t[jt][:, qt * 128:(qt + 1) * 128],
                        rhs=wo_sb[:, jt, nb * 512:(nb + 1) * 512],
                        start=(jt == 0),
                        stop=(jt == NJT - 1),
                    )
            ys = ypool.tile([128, 1024], BF16, tag="y", name="ys", bufs=2)
            nc.vector.tensor_copy(out=ys[:], in_=py[:])
            nc.scalar.dma_start(out=y[qt * 128:(qt + 1) * 128, :], in_=ys[:])
        return emit

    # filler schedule: per unit index, closures emitted inside its kt loop
    fillers = {
        0: [k_closure(1)], 1: [q_closure(1, 0)],
        2: [k_closure(2)], 3: [q_closure(2, 0)],
        4: [k_closure(3)], 5: [q_closure(3, 0)],
        6: [q_closure(0, 1)], 7: [q_closure(1, 1)],
        8: [q_closure(2, 1)], 9: [q_closure(3, 1)],
        10: [outproj(0)], 11: [outproj(1), outproj(2)],
        12: [outproj(3), outproj(4)], 13: [outproj(5), outproj(6)],
        14: [outproj(7)], 15: [],
    }

    # ---- attention: 16 units (q-half, head-pair, head) ----
    units = [(qh, jt, hh) for qh in range(2) for jt in range(NJT)
             for hh in range(2)]
    for ui, (qh, jt, hh) in enumerate(units):
        q0 = qh * 1024
        r0 = hh * 64
        h = 2 * jt + hh
        avp = ps_av(f"av{ui}")
        todo = list(fillers[ui])
        # duplicate this head's Q into the other partition half so the
        # odd/even row-tiled score matmuls both have matching-row operands
        qdup = smalls.tile([128, 1024], BF16, tag="qh", name="qdup", bufs=2)
        for half in range(2):
            nc.sync.dma_start(out=qdup[half * 64:half * 64 + 64, :],
                              in_=qt_t[jt][r0:r0 + 64, q0:q0 + 1024])
        for pi in range(NKT // 2):
            if todo and pi in (2, 5):
                todo.pop(0)()
            sTs = [ps_s(f"sT{ui}_{2 * pi + p}") for p in range(2)]
            for qbh in range(2):
                for par in range(2):
                    nc.tensor.matmul(
                        sTs[par][:, qbh * 512:(qbh + 1) * 512],
                        lhsT=kh_t[h][par * 64:par * 64 + 64, pi, :],
                        rhs=qdup[par * 64:par * 64 + 64,
                                 qbh * 512:(qbh + 1) * 512],
                        start=True,
                        stop=True,
                        tile_position=(par * 64, 0),
                    )
            for par in range(2):
                kt = 2 * pi + par
                sT = sTs[par]
                et = epool.tile([128, 1024], BF16, tag="e", name="et", bufs=3)
                if not _sch_tile(hh, kt):
                    nc.scalar.activation(et[:], sT[:], EXP, bias=ebias)
                else:
                    u1 = upool.tile([128, 1024], U16, tag="u1", name="u1",
                                    bufs=2)
                    u2 = upool.tile([128, 1024], U16, tag="u2", name="u2",
                                    bufs=2)
                    nc.vector.tensor_scalar(
                        out=u1[:], in0=sT[:], scalar1=float(SCH_A),
                        scalar2=float(SCH_B1), op0=MULT, op1=ADD)
                    nc.vector.tensor_scalar(
                        out=u2[:], in0=sT[:], scalar1=float(SCH_A),
                        scalar2=float(SCH_B2), op0=MULT, op1=ADD)
                    nc.gpsimd.tensor_add(
                        out=et[:], in0=u1.bitcast(BF16), in1=u2.bitcast(BF16))
                for qbh in range(2):
                    nc.tensor.matmul(
                        avp[0:DK + 1, qbh * 512:(qbh + 1) * 512],
                        lhsT=v_t[kt][:, h, 0:DK + 1],
                        rhs=et[:, qbh * 512:(qbh + 1) * 512],
                        start=(kt == 0),
                        stop=(kt == NKT - 1),
                    )
        while todo:
            todo.pop(0)()
        # softmax division: oc rows = this head's 64 dims
        avsb = smalls.tile([128, 1024], F32, tag="avsb", name="avsb", bufs=2)
        nc.vector.tensor_copy(out=avsb[0:DK + 1, :], in_=avp[0:DK + 1, :])
        sdram = dramp.tile([1024], F32, tag="sdram", name="sdram", bufs=2)
        nc.sync.dma_start(out=sdram[None, :], in_=avsb[DK:DK + 1, :])
        rsb = smalls.tile([128, 16], F32, tag="rsb", name="rsb", bufs=2)
        nc.sync.dma_start(out=rsb[0:64, :],
                          in_=sdram.rearrange("(p a) -> p a", p=64))
        rc2 = smalls.tile([128, 16], F32, tag="rc2", name="rc2", bufs=2)
        nc.vector.reciprocal(rc2[0:64, :], rsb[0:64, :])
        rdram = dramp.tile([1024], F32, tag="rdram", name="rdram", bufs=2)
        nc.sync.dma_start(out=rdram.rearrange("(p a) -> p a", p=64),
                          in_=rc2[0:64, :])
        rb = smalls.tile([128, 1024], F32, tag="rb", name="rb", bufs=2)
        nc.sync.dma_start(out=rb[0:DK, :],
                          in_=_bcast_rows(rdram[None, :], DK))
        nc.vector.scalar_tensor_tensor(
            out=oc_t[jt][r0:r0 + 64, q0:q0 + 1024],
            in0=avsb[0:DK, :], scalar=1.0, in1=rb[0:DK, :],
            op0=MULT, op1=MULT)

    # ---- remaining out-projection (qh1 rows) ----
    for qt in range(8, NKT):
        outproj(qt)()


_CACHE = {}


def _build():
    if "nc" in _CACHE:
        return _CACHE["nc"]
    from contextlib import ExitStack

    nc = bacc.Bacc("TRN2", target_bir_lowering=False, debug=False,
                   num_devices=NCORES)
    with tile.TileContext(nc) as tc:
        with ExitStack() as ctx:
            _emit(nc, tc, ctx)
    nc.compile()
    _CACHE["nc"] = nc
    return nc


def make_in_maps(query, key, value, Wq, bq, Wk, bk, Wv, bv, Wo, bo):
    import ml_dtypes
    bf = ml_dtypes.bfloat16
    f32 = np.float32
    query = np.asarray(query, f32)
    key = np.asarray(key, f32)
    value = np.asarray(value, f32)
    Wq, Wk, Wv, Wo = (np.asarray(a, f32) for a in (Wq, Wk, Wv, Wo))
    bq, bk, bv = (np.asarray(a, f32) for a in (bq, bk, bv))
    scale = f32(1.0 / np.sqrt(DK))
    xT = {}
    for b in range(B):
        xT[b] = (
            np.ascontiguousarray(query[b].T.astype(bf)),
            np.ascontiguousarray(key[b].T.astype(bf)),
            np.ascontiguousarray(value[b].T.astype(bf)),
        )
    in_maps = []
    for c in range(NCORES):
        b, hh = divmod(c, 2)
        js = slice(hh * DH, (hh + 1) * DH)
        xqT, xkT, xvT = xT[b]
        in_maps.append({
            "xq": xqT,
            "xk": xkT,
            "xv": xvT,
            "wq": np.ascontiguousarray((Wq[:, js] * scale).astype(bf)),
            "bq": np.ascontiguousarray(bq[js] * scale),
            "wk": np.ascontiguousarray(Wk[:, js].astype(bf)),
            "bk": np.ascontiguousarray(bk[js]),
            "wv": np.ascontiguousarray(Wv[:, js].astype(bf)),
            "bv": np.ascontiguousarray(bv[js]),
            "wo": np.ascontiguousarray(Wo[js, :].astype(bf)),
        })
    return in_maps


LAST_RESULTS = None


def kernel(query, key, value, Wq, bq, Wk, bk, Wv, bv, Wo, bo):
    global LAST_RESULTS
    import os
    from concourse.bass_utils import run_bass_kernel_spmd

    nc = _build()
    in_maps = make_in_maps(query, key, value, Wq, bq, Wk, bk, Wv, bv, Wo, bo)
    trace = bool(int(os.environ.get("KERNEL_TRACE", "0")))
    res = run_bass_kernel_spmd(nc, in_maps, list(range(NCORES)), trace=trace)
    LAST_RESULTS = res
    bo32 = np.asarray(bo, dtype=np.float32)
    out = np.empty((B, S, D), dtype=np.float32)
    for b in range(B):
        out[b] = (res.results[2 * b]["y"].astype(np.float32)
                  + res.results[2 * b + 1]["y"].astype(np.float32) + bo32)
    return out
